# revision 48
# baseline (speedup 1.0000x reference)
# MoE (top-2 of 8 experts) Trainium2 kernel.
#
# Strategy — hidden-dimension (H) slicing across all 8 cores:
#   - Gate (softmax + top-2 + renormalize) computed on host in f32 — it is
#     0.006% of the FLOPs and produces the data-dependent routing needed to
#     lay tokens out by expert.
#   - EVERY core processes ALL 16384 token-expert assignments, but each core
#     computes only a 512-wide slice of the hidden dimension H=4096 (core k
#     owns h in [512k, 512k+512)).  relu splits cleanly across H, so the full
#     FFN output is the elementwise sum of the 8 per-core partials; the host
#     sums them during unshard.  Per-core work is exactly 1/8 of the total
#     independent of routing — perfect load balance by construction, and one
#     uniform SPMD program (all cores even share the same x input).
#   - Device per token chunk (<=512 tokens, tokens on the matmul free dim):
#       phase 1:  h[g]   = relu( w1[kc,:,g*128:+128].T @ xT[kc] + b1 ), kc=0..7
#       phase 2:  out[mc] =      w2[kh,:,mc*128:+128].T @ h[kh],        kh=0..3
#     All 8 experts' H-slice weights stay resident in SBUF (bf16, 16 MB);
#     token chunks stream through on a dedicated DMA queue; bf16 partial
#     outputs stream back on another.
#   - DMA choreography (all measured on HW): the DMA subsystem ramps slowly
#     for the first ~15-20us, so warm-up matmuls on a memset tile keep the
#     PE busy (and its HAM clock-gate at full 2.4 GHz) while the lead
#     transfers land, and the first chunks are narrow and staircased with
#     w1 quarter-tiles across the two HWDGE queues (sync/scalar).  Weight
#     packs for later experts are emitted lazily so the sync FIFO paces
#     deliveries with consumption.  Out-stores ride the gpsimd SWDGE queue
#     (an HWDGE store while weights stream shares completion-sem lanes with
#     them, and a blocked store trigger head-of-line-blocks the relus);
#     only the last few chunks' stores use sync/scalar so no SWDGE backlog
#     delays the final drain.

import os
import sys
import types

import numpy as np
import ml_dtypes

P = 128
C = 1024
H = 4096
H8 = H // 8     # per-core H slice: 512
E = 8
N_CORES = 8
KC = C // P     # 8
KH = H8 // P    # 4
MAXW = 512      # max tokens per chunk (PSUM bank: 512 f32)
LEADWS = (128, 384)   # first chunks: narrow, staircased with the DMA ramp
TAILW = 192     # last chunk processed: narrow, so the final drain is short
NWARM = 32      # prologue warm matmuls: spans the ~8-18us DMA-subsystem ramp
BF16 = ml_dtypes.bfloat16

TRACE = bool(int(os.environ.get("KERNEL_TRACE", "0")))
LAST_EXEC_NS = None
LAST_RESULTS = None


def _ensure_axon_hooks_shim():
    """bass_utils imports antenv.axon_hooks when tracing is requested; this
    image's antenv lacks that module. Provide it, backed by the axon PJRT .so
    profiling C ABI when available."""
    try:
        import antenv.axon_hooks  # noqa: F401
        return
    except ImportError:
        pass
    mod = types.ModuleType("antenv.axon_hooks")
    mod._hook = None

    def set_axon_ntff_profile_hook(h):
        mod._hook = h

    def get_axon_ntff_profile_hook():
        return mod._hook

    mod.set_axon_ntff_profile_hook = set_axon_ntff_profile_hook
    mod.get_axon_ntff_profile_hook = get_axon_ntff_profile_hook
    try:
        import antenv
        sys.modules["antenv.axon_hooks"] = mod
        antenv.axon_hooks = mod
    except ImportError:
        antenv = types.ModuleType("antenv")
        antenv.axon_hooks = mod
        sys.modules["antenv"] = antenv
        sys.modules["antenv.axon_hooks"] = mod
    try:
        from trn_agent_boot.trn_boot import _ntff_profile_via_ctypes
        h = _ntff_profile_via_ctypes("/opt/axon/libaxon_pjrt.so")
        if h is not None:
            mod._hook = h
    except Exception:
        pass


_COMPILED = {}


def _equal_chunks(n):
    if n <= 0:
        return []
    k = -(-n // MAXW)
    q, r = divmod(n, k)
    return [q + 1] * r + [q] * (k - r)


def _build(plans):
    import concourse.mybir as mybir
    import concourse.tile as tile
    from concourse import bacc

    f32 = mybir.dt.float32
    bf16 = mybir.dt.bfloat16

    NT = sum(sum(p) for p in plans)

    nc = bacc.Bacc("TRN2", target_bir_lowering=False, debug=False,
                   num_devices=N_CORES)

    xt_d = nc.dram_tensor("xt", [P, KC * NT], bf16, kind="ExternalInput")
    w1_d = nc.dram_tensor("w1r", [P, E * KC * H8], bf16, kind="ExternalInput")
    w2_d = nc.dram_tensor("w2r", [P, E * KH * C], bf16, kind="ExternalInput")
    b1_d = nc.dram_tensor("b1r", [P, E * KH], f32, kind="ExternalInput")
    out_d = nc.dram_tensor("outp", [P, KC * NT], bf16, kind="ExternalOutput")

    # partition-major views (host pre-lays everything partition-major, so
    # every DMA walks HBM near-sequentially)
    x_t = xt_d.ap().rearrange("p (kc n) -> p kc n", kc=KC)
    w1_t = w1_d.ap().rearrange("p (e kc h) -> p e kc h", e=E, kc=KC)
    w2_t = w2_d.ap().rearrange("p (e kh c) -> p e kh c", e=E, kh=KH)
    out_t = out_d.ap().rearrange("p (mc n) -> p mc n", mc=KC)

    relu = mybir.ActivationFunctionType.Relu

    nchunks_total = sum(len(p) for p in plans)

    with tile.TileContext(nc) as tc:
        with (
            tc.tile_pool(name="wres", bufs=1) as wpool,
            tc.tile_pool(name="bias", bufs=1) as bpool,
            tc.tile_pool(name="xin", bufs=3) as xpool,
            tc.tile_pool(name="hmid", bufs=3) as hpool,
            tc.tile_pool(name="oout", bufs=3) as opool,
            tc.tile_pool(name="ps1", bufs=4, space="PSUM") as ps1pool,
            tc.tile_pool(name="ps2", bufs=4, space="PSUM") as ps2pool,
        ):
            # bias heads the sync queue: tiny, needed by the first relu
            b1_sb = bpool.tile([P, E * KH], f32, tag="b1")
            nc.sync.dma_start(b1_sb[:], b1_d.ap())

            # warm-up matmuls on a memset tile: keep the PE busy (and the
            # HAM clock gate at full rate) while the DMA subsystem ramps;
            # also woven as filler between the first chunks' groups
            warm_sb = bpool.tile([P, 640], bf16, tag="warm")
            nc.vector.memset(warm_sb[:], 0.5)

            def warm(n):
                wps = ps1pool.tile([P, MAXW], f32, tag="ps1", name="wps")
                for i in range(n):
                    nc.tensor.matmul(wps[:], warm_sb[:, 0:P],
                                     warm_sb[:, P:640],
                                     start=(i == 0), stop=(i == n - 1))

            warm(NWARM)

            # first token chunk (narrow lead) on sync: lands in ~1us
            W0 = plans[0][0]
            x_first = xpool.tile([P, KC * W0], bf16, tag="x")
            nc.sync.dma_start(
                x_first[:].rearrange("p (kc w) -> p kc w", kc=KC),
                x_t[:, :, 0:W0])

            # global chunk schedule
            sched = []
            off = 0
            for e in range(E):
                for W in plans[e]:
                    sched.append((e, off, W))
                    off += W

            # early x chunks prefetched on HWDGE queues, interleaved with
            # expert 0's weight halves in consumption order; chunk 3+ rides
            # gpsimd, whose ~8-10us latency is hidden by then
            x_pre = {0: x_first}

            def prefetch_x(i, eng):
                e_, off_, W_ = sched[i]
                t = xpool.tile([P, KC * W_], bf16, tag="x", name="x_sb")
                eng.dma_start(t[:].rearrange("p (kc w) -> p kc w", kc=KC),
                              x_t[:, :, off_:off_ + W_])
                x_pre[i] = t

            def w1_load(e, q, eng):
                # quarter-tiles: one phase-1 group's weights per transfer,
                # so the startup staircase unlocks one group per DMA slot
                t = wpool.tile([P, KC * P], bf16, tag=f"w1_{e}_{q}",
                               name="t")
                eng.dma_start(t[:].rearrange("p (kc h) -> p kc h", kc=KC),
                              w1_t[:, e, :, q * P:(q + 1) * P])
                w1_sb[(e, q)] = t

            def w2_load(e, eng0, eng1):
                t = wpool.tile([P, KH * C], bf16, tag=f"w2_{e}", name="t")
                t_v = t[:].rearrange("p (kh c) -> p kh c", kh=KH)
                eng0.dma_start(t_v[:, :, 0:C // 2], w2_t[:, e, :, 0:C // 2])
                eng1.dma_start(t_v[:, :, C // 2:C], w2_t[:, e, :, C // 2:C])
                w2_sb[e] = t

            w1_sb, w2_sb = {}, {}
            # expert 0 + lead x chunks: arrival order (one transfer per
            # queue per DMA-ramp slot) matched to consumption order
            # sync:   b1, x0, q1, q3, w2h0, e1+e2 weights, stores|weights...
            # scalar: q0, x1, q2, w2h1, late stores
            # gpsimd: x2, x3, x4, ... (x only — a store ahead of an x chunk
            #         on the SWDGE FIFO delays it by ~10us)
            w1_load(0, 0, nc.scalar)
            w1_load(0, 1, nc.sync)
            if len(sched) > 1:
                prefetch_x(1, nc.scalar)
            w1_load(0, 2, nc.scalar)
            w1_load(0, 3, nc.sync)
            w2_load(0, nc.sync, nc.scalar)
            if len(sched) > 2:
                prefetch_x(2, nc.gpsimd)

            def wexp_load(e):
                for q in range(KH):
                    w1_load(e, q, nc.sync)
                w2_load(e, nc.sync, nc.sync)

            # experts 1-2 up front; 3+ are emitted lazily at expert
            # boundaries so the sync FIFO paces weight deliveries with
            # consumption and early out-stores slot in between them
            for e in range(1, min(3, E)):
                wexp_load(e)

            def phase1(e, off, W, x_sb, filler=()):
                if x_sb is None:
                    x_sb = xpool.tile([P, KC * W], bf16, tag="x", name="x_sb")
                    nc.gpsimd.dma_start(
                        x_sb[:].rearrange("p (kc w) -> p kc w", kc=KC),
                        x_t[:, :, off:off + W])
                h_tiles = []
                for g in range(KH):
                    wt = w1_sb[(e, g)]
                    ps = ps1pool.tile([P, W], f32, tag="ps1", name="ps")
                    for kc in range(KC):
                        nc.tensor.matmul(
                            ps[:],
                            wt[:, kc * P:(kc + 1) * P],
                            x_sb[:, kc * W:(kc + 1) * W],
                            start=(kc == 0),
                            stop=(kc == KC - 1),
                        )
                    ht = hpool.tile([P, W], bf16, tag=f"h_{g}", name="ht")
                    nc.scalar.activation(
                        ht[:], ps[:], relu,
                        bias=b1_sb[:, e * KH + g:e * KH + g + 1],
                        scale=1.0)
                    h_tiles.append(ht)
                    if filler and g < len(filler):
                        warm(filler[g])
                return h_tiles

            def phase2(e, off, W, h_tiles, idx):
                # out-stores ride HWDGE: scalar while the sync queue is
                # still streaming weights, alternating scalar/sync after;
                # last chunk: eight narrow stores so the final drain is
                # short.  (gpsimd SWDGE stores fall ~10us/transfer behind
                # and the end-of-kernel barrier waits for the backlog.)
                last = (idx == len(sched) - 1)
                nsplit = 4 if last else 1
                mc_per = KC // nsplit
                for sp in range(nsplit):
                    otag = f"ol_{sp}" if last else "o"
                    o_sb = opool.tile([P, mc_per * W], bf16, tag=otag,
                                      bufs=1 if last else None, name="o_sb")
                    for mci in range(mc_per):
                        mc = sp * mc_per + mci
                        ps = ps2pool.tile([P, W], f32, tag="ps2", name="ps")
                        for kh in range(KH):
                            nc.tensor.matmul(
                                ps[:],
                                w2_sb[e][:, kh * C + mc * P:
                                         kh * C + mc * P + P],
                                h_tiles[kh][:],
                                start=(kh == 0),
                                stop=(kh == KH - 1),
                            )
                        nc.vector.tensor_copy(
                            o_sb[:, mci * W:(mci + 1) * W], ps[:])
                    # stores ride gpsimd SWDGE while any weights may still
                    # be streaming — an HWDGE store in that era shares
                    # completion-sem lanes with the weight transfers and a
                    # blocked store trigger stalls the relus behind it.
                    # Only the final chunks alternate sync/scalar HWDGE, so
                    # no SWDGE backlog delays the final drain.
                    if last:
                        eng = nc.sync if sp % 2 else nc.scalar
                    elif idx < len(sched) - 6:
                        eng = nc.gpsimd
                    else:
                        eng = nc.sync if idx % 2 else nc.scalar
                    eng.dma_start(
                        out_t[:, sp * mc_per:(sp + 1) * mc_per,
                              off:off + W],
                        o_sb[:].rearrange("p (mc w) -> p mc w", mc=mc_per))

            # software-pipeline the first three chunks (their phase-1s run
            # before any phase-2) so expert 0's w2 is not needed until two
            # chunk-times later, with warm filler between the staircase
            # groups — the DMA ramp delivers ~one transfer per queue per
            # ~4us early on, far slower than the PE consumes
            npipe = min(3, len(sched))
            hs = [phase1(*sched[i], x_pre.get(i),
                         filler=(4, 4, 4) if i == 0 else ())
                  for i in range(npipe)]
            for i in range(npipe):
                phase2(*sched[i], hs[i], i)
            next_pack = 3
            cur_e = sched[0][0]
            for i in range(npipe, len(sched)):
                e_i = sched[i][0]
                if e_i != cur_e:
                    cur_e = e_i
                    while next_pack <= min(e_i + 2, E - 1):
                        wexp_load(next_pack)
                        next_pack += 1
                h = phase1(*sched[i], x_pre.get(i))
                phase2(*sched[i], h, i)

    nc.compile()
    return nc


def _get_compiled(plans):
    key = plans
    if key not in _COMPILED:
        _COMPILED[key] = _build(plans)
    return _COMPILED[key]


def kernel(x, gate_w, w1, b1, w2, b2):
    global LAST_EXEC_NS, LAST_RESULTS
    _ensure_axon_hooks_shim()
    from concourse import bass_utils

    B, T, _ = x.shape
    N = B * T
    xf = np.ascontiguousarray(x.reshape(N, C)).astype(np.float32, copy=False)

    # --- gate on host (f32, matches reference numerics) ---
    logits = xf @ np.ascontiguousarray(gate_w.astype(np.float32)).T
    m = logits.max(axis=1, keepdims=True)
    ew = np.exp(logits - m)
    sw = ew / ew.sum(axis=1, keepdims=True)        # [N, E] f32 softmax
    ar = np.arange(N)
    i0 = sw.argmax(axis=1)
    w0 = sw[ar, i0]
    swm = sw.copy()
    swm[ar, i0] = -1.0
    i1 = swm.argmax(axis=1)
    w1g = sw[ar, i1]
    tot = w0 + w1g
    cw0 = (w0 / tot).astype(np.float32)
    cw1 = (w1g / tot).astype(np.float32)

    # --- dispatch: token lists per expert, concatenated ---
    idx_list, cw_list = [], []
    for e in range(E):
        s0 = i0 == e
        s1 = i1 == e
        idx_list.append(np.concatenate([ar[s0], ar[s1]]))
        cw_list.append(np.concatenate([cw0[s0], cw1[s1]]).astype(np.float32))
    counts = [len(ix) for ix in idx_list]
    NT = sum(counts)

    plans = []
    lead = sum(LEADWS)
    for e, n in enumerate(counts):
        if e == 0 and n > 3 * lead:
            plan = list(LEADWS) + _equal_chunks(n - lead)
        elif e == E - 1 and n > 3 * TAILW + lead:
            plan = _equal_chunks(n - TAILW) + [TAILW]
        else:
            plan = _equal_chunks(n)
        plans.append(tuple(plan))
    plans = tuple(plans)
    nc = _get_compiled(plans)

    # --- per-core inputs (host lays everything partition-major) ---
    idx_all = np.concatenate(idx_list)
    xg = xf[idx_all].astype(BF16)                       # [NT, C]
    xt = np.ascontiguousarray(
        xg.reshape(NT, KC, P).transpose(2, 1, 0)).reshape(P, KC * NT)

    w1b = w1.astype(BF16)                               # [E, C, H]
    w2b = w2.astype(BF16)                               # [E, H, C]
    b1f = b1.astype(np.float32)

    in_maps = []
    for k in range(N_CORES):
        hs = slice(k * H8, (k + 1) * H8)
        w1r = np.ascontiguousarray(
            w1b[:, :, hs].reshape(E, KC, P, H8)
            .transpose(2, 0, 1, 3)).reshape(P, E * KC * H8)
        w2r = np.ascontiguousarray(
            w2b[:, hs, :].reshape(E, KH, P, C)
            .transpose(2, 0, 1, 3)).reshape(P, E * KH * C)
        b1r = np.ascontiguousarray(
            b1f[:, hs].reshape(E, KH, P).transpose(2, 0, 1)).reshape(P, E * KH)
        in_maps.append({"xt": xt, "w1r": w1r, "w2r": w2r, "b1r": b1r})

    try:
        res = bass_utils.run_bass_kernel_spmd(
            nc, in_maps, core_ids=list(range(N_CORES)), trace=TRACE)
    except Exception:
        if not TRACE:
            raise
        # profiling plumbing can fail in restricted environments — the
        # numerical result must not depend on it
        res = bass_utils.run_bass_kernel_spmd(
            nc, in_maps, core_ids=list(range(N_CORES)), trace=False)
    LAST_RESULTS = res
    LAST_EXEC_NS = res.exec_time_ns

    # --- combine (host unshard): sum the 8 H-slice partials, add b2,
    #     apply gate combine weights ---
    acc = np.zeros((P, KC, NT), dtype=np.float32)
    for k in range(N_CORES):
        acc += res.results[k]["outp"].reshape(P, KC, NT)
    y = np.ascontiguousarray(acc.transpose(2, 1, 0)).reshape(NT, C)

    out = np.zeros((N, C), dtype=np.float32)
    b2f = b2.astype(np.float32)
    pos = 0
    for e in range(E):
        n_e = counts[e]
        seg = y[pos:pos + n_e]
        seg += b2f[e][None, :]
        out[idx_list[e]] += cw_list[e][:, None] * seg
        pos += n_e
    return out.reshape(B, T, C).astype(x.dtype, copy=False)


# revision 49
# speedup vs baseline: 1.0101x; 1.0101x over previous
# MoE (top-2 of 8 experts) Trainium2 kernel.
#
# Strategy — hidden-dimension (H) slicing across all 8 cores:
#   - Gate (softmax + top-2 + renormalize) computed on host in f32 — it is
#     0.006% of the FLOPs and produces the data-dependent routing needed to
#     lay tokens out by expert.
#   - EVERY core processes ALL 16384 token-expert assignments, but each core
#     computes only a 512-wide slice of the hidden dimension H=4096 (core k
#     owns h in [512k, 512k+512)).  relu splits cleanly across H, so the full
#     FFN output is the elementwise sum of the 8 per-core partials; the host
#     sums them during unshard.  Per-core work is exactly 1/8 of the total
#     independent of routing — perfect load balance by construction, and one
#     uniform SPMD program (all cores even share the same x input).
#   - Device per token chunk (<=512 tokens, tokens on the matmul free dim):
#       phase 1:  h[g]   = relu( w1[kc,:,g*128:+128].T @ xT[kc] + b1 ), kc=0..7
#       phase 2:  out[mc] =      w2[kh,:,mc*128:+128].T @ h[kh],        kh=0..3
#     All 8 experts' H-slice weights stay resident in SBUF (bf16, 16 MB);
#     token chunks stream through on a dedicated DMA queue; bf16 partial
#     outputs stream back on another.
#   - DMA choreography (all measured on HW): the DMA subsystem ramps slowly
#     for the first ~15-20us, so warm-up matmuls on a memset tile keep the
#     PE busy (and its HAM clock-gate at full 2.4 GHz) while the lead
#     transfers land, and the first chunks are narrow and staircased with
#     w1 quarter-tiles across the two HWDGE queues (sync/scalar).  Weight
#     packs for later experts are emitted lazily so the sync FIFO paces
#     deliveries with consumption.  Out-stores ride the gpsimd SWDGE queue
#     (an HWDGE store while weights stream shares completion-sem lanes with
#     them, and a blocked store trigger head-of-line-blocks the relus);
#     only the last few chunks' stores use sync/scalar so no SWDGE backlog
#     delays the final drain.

import os
import sys
import types

import numpy as np
import ml_dtypes

P = 128
C = 1024
H = 4096
H8 = H // 8     # per-core H slice: 512
E = 8
N_CORES = 8
KC = C // P     # 8
KH = H8 // P    # 4
MAXW = 512      # max tokens per chunk (PSUM bank: 512 f32)
LEADWS = (128, 384)   # first chunks: narrow, staircased with the DMA ramp
TAILW = 192     # last chunk processed: narrow, so the final drain is short
NWARM = 32      # prologue warm matmuls: spans the ~8-18us DMA-subsystem ramp
BF16 = ml_dtypes.bfloat16

TRACE = bool(int(os.environ.get("KERNEL_TRACE", "0")))
LAST_EXEC_NS = None
LAST_RESULTS = None


def _ensure_axon_hooks_shim():
    """bass_utils imports antenv.axon_hooks when tracing is requested; this
    image's antenv lacks that module. Provide it, backed by the axon PJRT .so
    profiling C ABI when available."""
    try:
        import antenv.axon_hooks  # noqa: F401
        return
    except ImportError:
        pass
    mod = types.ModuleType("antenv.axon_hooks")
    mod._hook = None

    def set_axon_ntff_profile_hook(h):
        mod._hook = h

    def get_axon_ntff_profile_hook():
        return mod._hook

    mod.set_axon_ntff_profile_hook = set_axon_ntff_profile_hook
    mod.get_axon_ntff_profile_hook = get_axon_ntff_profile_hook
    try:
        import antenv
        sys.modules["antenv.axon_hooks"] = mod
        antenv.axon_hooks = mod
    except ImportError:
        antenv = types.ModuleType("antenv")
        antenv.axon_hooks = mod
        sys.modules["antenv"] = antenv
        sys.modules["antenv.axon_hooks"] = mod
    try:
        from trn_agent_boot.trn_boot import _ntff_profile_via_ctypes
        h = _ntff_profile_via_ctypes("/opt/axon/libaxon_pjrt.so")
        if h is not None:
            mod._hook = h
    except Exception:
        pass


_COMPILED = {}


def _equal_chunks(n):
    if n <= 0:
        return []
    k = -(-n // MAXW)
    q, r = divmod(n, k)
    return [q + 1] * r + [q] * (k - r)


def _build(plans):
    import concourse.mybir as mybir
    import concourse.tile as tile
    from concourse import bacc

    f32 = mybir.dt.float32
    bf16 = mybir.dt.bfloat16

    NT = sum(sum(p) for p in plans)

    nc = bacc.Bacc("TRN2", target_bir_lowering=False, debug=False,
                   num_devices=N_CORES)

    xt_d = nc.dram_tensor("xt", [P, KC * NT], bf16, kind="ExternalInput")
    w1_d = nc.dram_tensor("w1r", [P, E * KC * H8], bf16, kind="ExternalInput")
    w2_d = nc.dram_tensor("w2r", [P, E * KH * C], bf16, kind="ExternalInput")
    b1_d = nc.dram_tensor("b1r", [P, E * KH], f32, kind="ExternalInput")
    out_d = nc.dram_tensor("outp", [P, KC * NT], bf16, kind="ExternalOutput")

    # partition-major views (host pre-lays everything partition-major, so
    # every DMA walks HBM near-sequentially)
    x_t = xt_d.ap().rearrange("p (kc n) -> p kc n", kc=KC)
    w1_t = w1_d.ap().rearrange("p (e kc h) -> p e kc h", e=E, kc=KC)
    w2_t = w2_d.ap().rearrange("p (e kh c) -> p e kh c", e=E, kh=KH)
    out_t = out_d.ap().rearrange("p (mc n) -> p mc n", mc=KC)

    relu = mybir.ActivationFunctionType.Relu

    nchunks_total = sum(len(p) for p in plans)

    with tile.TileContext(nc) as tc:
        with (
            tc.tile_pool(name="wres", bufs=1) as wpool,
            tc.tile_pool(name="bias", bufs=1) as bpool,
            tc.tile_pool(name="xin", bufs=3) as xpool,
            tc.tile_pool(name="hmid", bufs=2) as hpool,
            tc.tile_pool(name="oout", bufs=3) as opool,
            tc.tile_pool(name="ps1", bufs=4, space="PSUM") as ps1pool,
            tc.tile_pool(name="ps2", bufs=4, space="PSUM") as ps2pool,
        ):
            # bias heads the sync queue: tiny, needed by the first relu
            b1_sb = bpool.tile([P, E * KH], f32, tag="b1")
            nc.sync.dma_start(b1_sb[:], b1_d.ap())

            # warm-up matmuls on a memset tile: keep the PE busy (and the
            # HAM clock gate at full rate) while the DMA subsystem ramps;
            # also woven as filler between the first chunks' groups
            warm_sb = bpool.tile([P, 640], bf16, tag="warm")
            nc.vector.memset(warm_sb[:], 0.5)

            def warm(n):
                wps = ps1pool.tile([P, MAXW], f32, tag="ps1", name="wps")
                for i in range(n):
                    nc.tensor.matmul(wps[:], warm_sb[:, 0:P],
                                     warm_sb[:, P:640],
                                     start=(i == 0), stop=(i == n - 1))

            warm(NWARM)

            # first token chunk (narrow lead) on sync: lands in ~1us
            W0 = plans[0][0]
            x_first = xpool.tile([P, KC * W0], bf16, tag="x")
            nc.sync.dma_start(
                x_first[:].rearrange("p (kc w) -> p kc w", kc=KC),
                x_t[:, :, 0:W0])

            # global chunk schedule
            sched = []
            off = 0
            for e in range(E):
                for W in plans[e]:
                    sched.append((e, off, W))
                    off += W

            # early x chunks prefetched on HWDGE queues, interleaved with
            # expert 0's weight halves in consumption order; chunk 3+ rides
            # gpsimd, whose ~8-10us latency is hidden by then
            x_pre = {0: x_first}

            def prefetch_x(i, eng):
                e_, off_, W_ = sched[i]
                t = xpool.tile([P, KC * W_], bf16, tag="x", name="x_sb")
                eng.dma_start(t[:].rearrange("p (kc w) -> p kc w", kc=KC),
                              x_t[:, :, off_:off_ + W_])
                x_pre[i] = t

            def w1_load(e, q, eng):
                # quarter-tiles: one phase-1 group's weights per transfer,
                # so the startup staircase unlocks one group per DMA slot
                t = wpool.tile([P, KC * P], bf16, tag=f"w1_{e}_{q}",
                               name="t")
                eng.dma_start(t[:].rearrange("p (kc h) -> p kc h", kc=KC),
                              w1_t[:, e, :, q * P:(q + 1) * P])
                w1_sb[(e, q)] = t

            def w2_load(e, eng0, eng1):
                t = wpool.tile([P, KH * C], bf16, tag=f"w2_{e}", name="t")
                t_v = t[:].rearrange("p (kh c) -> p kh c", kh=KH)
                eng0.dma_start(t_v[:, :, 0:C // 2], w2_t[:, e, :, 0:C // 2])
                eng1.dma_start(t_v[:, :, C // 2:C], w2_t[:, e, :, C // 2:C])
                w2_sb[e] = t

            w1_sb, w2_sb = {}, {}
            # expert 0 + lead x chunks: arrival order (one transfer per
            # queue per DMA-ramp slot) matched to consumption order
            # sync:   b1, x0, q1, q3, w2h0, e1+e2 weights, stores|weights...
            # scalar: q0, x1, q2, w2h1, late stores
            # gpsimd: x2, x3, x4, ... (x only — a store ahead of an x chunk
            #         on the SWDGE FIFO delays it by ~10us)
            w1_load(0, 0, nc.scalar)
            w1_load(0, 1, nc.sync)
            if len(sched) > 1:
                prefetch_x(1, nc.scalar)
            w1_load(0, 2, nc.scalar)
            w1_load(0, 3, nc.sync)
            w2_load(0, nc.sync, nc.scalar)
            if len(sched) > 2:
                prefetch_x(2, nc.gpsimd)

            def wexp_load(e):
                for q in range(KH):
                    w1_load(e, q, nc.sync)
                w2_load(e, nc.sync, nc.sync)

            # experts 1-2 up front; 3+ are emitted lazily at expert
            # boundaries so the sync FIFO paces weight deliveries with
            # consumption and early out-stores slot in between them
            for e in range(1, min(3, E)):
                wexp_load(e)

            def phase1(e, off, W, x_sb, filler=()):
                if x_sb is None:
                    x_sb = xpool.tile([P, KC * W], bf16, tag="x", name="x_sb")
                    nc.gpsimd.dma_start(
                        x_sb[:].rearrange("p (kc w) -> p kc w", kc=KC),
                        x_t[:, :, off:off + W])
                h_tiles = []
                for g in range(KH):
                    wt = w1_sb[(e, g)]
                    ps = ps1pool.tile([P, W], f32, tag="ps1", name="ps")
                    for kc in range(KC):
                        nc.tensor.matmul(
                            ps[:],
                            wt[:, kc * P:(kc + 1) * P],
                            x_sb[:, kc * W:(kc + 1) * W],
                            start=(kc == 0),
                            stop=(kc == KC - 1),
                        )
                    ht = hpool.tile([P, W], bf16, tag=f"h_{g}", name="ht")
                    nc.scalar.activation(
                        ht[:], ps[:], relu,
                        bias=b1_sb[:, e * KH + g:e * KH + g + 1],
                        scale=1.0)
                    h_tiles.append(ht)
                    if filler and g < len(filler):
                        warm(filler[g])
                return h_tiles

            def phase2(e, off, W, h_tiles, idx):
                # out-stores ride HWDGE: scalar while the sync queue is
                # still streaming weights, alternating scalar/sync after;
                # last chunk: eight narrow stores so the final drain is
                # short.  (gpsimd SWDGE stores fall ~10us/transfer behind
                # and the end-of-kernel barrier waits for the backlog.)
                last = (idx == len(sched) - 1)
                nsplit = 4 if last else 1
                mc_per = KC // nsplit
                for sp in range(nsplit):
                    otag = f"ol_{sp}" if last else "o"
                    o_sb = opool.tile([P, mc_per * W], bf16, tag=otag,
                                      bufs=1 if last else None, name="o_sb")
                    for mci in range(mc_per):
                        mc = sp * mc_per + mci
                        ps = ps2pool.tile([P, W], f32, tag="ps2", name="ps")
                        for kh in range(KH):
                            nc.tensor.matmul(
                                ps[:],
                                w2_sb[e][:, kh * C + mc * P:
                                         kh * C + mc * P + P],
                                h_tiles[kh][:],
                                start=(kh == 0),
                                stop=(kh == KH - 1),
                            )
                        nc.vector.tensor_copy(
                            o_sb[:, mci * W:(mci + 1) * W], ps[:])
                    # stores ride gpsimd SWDGE while any weights may still
                    # be streaming — an HWDGE store in that era shares
                    # completion-sem lanes with the weight transfers and a
                    # blocked store trigger stalls the relus behind it.
                    # Only the final chunks alternate sync/scalar HWDGE, so
                    # no SWDGE backlog delays the final drain.
                    if last:
                        eng = nc.sync if sp % 2 else nc.scalar
                    elif idx < len(sched) - 6:
                        eng = nc.gpsimd
                    else:
                        eng = nc.sync if idx % 2 else nc.scalar
                    eng.dma_start(
                        out_t[:, sp * mc_per:(sp + 1) * mc_per,
                              off:off + W],
                        o_sb[:].rearrange("p (mc w) -> p mc w", mc=mc_per))

            # software-pipeline the first two chunks (both phase-1s run
            # before either phase-2) so expert 0's w2 is not needed until a
            # full chunk-time later, with warm filler between the staircase
            # groups — the DMA ramp delivers ~one transfer per queue per
            # ~4us early on, far slower than the PE consumes
            h0 = phase1(*sched[0], x_pre.get(0), filler=(4, 4, 4))
            h1 = phase1(*sched[1], x_pre.get(1)) if len(sched) > 1 else None
            phase2(*sched[0], h0, 0)
            if h1 is not None:
                phase2(*sched[1], h1, 1)
            next_pack = 3
            cur_e = sched[0][0]
            for i in range(2, len(sched)):
                e_i = sched[i][0]
                if e_i != cur_e:
                    cur_e = e_i
                    while next_pack <= min(e_i + 2, E - 1):
                        wexp_load(next_pack)
                        next_pack += 1
                h = phase1(*sched[i], x_pre.get(i))
                phase2(*sched[i], h, i)

    nc.compile()
    return nc


def _get_compiled(plans):
    key = plans
    if key not in _COMPILED:
        _COMPILED[key] = _build(plans)
    return _COMPILED[key]


def kernel(x, gate_w, w1, b1, w2, b2):
    global LAST_EXEC_NS, LAST_RESULTS
    _ensure_axon_hooks_shim()
    from concourse import bass_utils

    B, T, _ = x.shape
    N = B * T
    xf = np.ascontiguousarray(x.reshape(N, C)).astype(np.float32, copy=False)

    # --- gate on host (f32, matches reference numerics) ---
    logits = xf @ np.ascontiguousarray(gate_w.astype(np.float32)).T
    m = logits.max(axis=1, keepdims=True)
    ew = np.exp(logits - m)
    sw = ew / ew.sum(axis=1, keepdims=True)        # [N, E] f32 softmax
    ar = np.arange(N)
    i0 = sw.argmax(axis=1)
    w0 = sw[ar, i0]
    swm = sw.copy()
    swm[ar, i0] = -1.0
    i1 = swm.argmax(axis=1)
    w1g = sw[ar, i1]
    tot = w0 + w1g
    cw0 = (w0 / tot).astype(np.float32)
    cw1 = (w1g / tot).astype(np.float32)

    # --- dispatch: token lists per expert, concatenated ---
    idx_list, cw_list = [], []
    for e in range(E):
        s0 = i0 == e
        s1 = i1 == e
        idx_list.append(np.concatenate([ar[s0], ar[s1]]))
        cw_list.append(np.concatenate([cw0[s0], cw1[s1]]).astype(np.float32))
    counts = [len(ix) for ix in idx_list]
    NT = sum(counts)

    plans = []
    lead = sum(LEADWS)
    for e, n in enumerate(counts):
        if e == 0 and n > 3 * lead:
            plan = list(LEADWS) + _equal_chunks(n - lead)
        elif e == E - 1 and n > 3 * TAILW + lead:
            plan = _equal_chunks(n - TAILW) + [TAILW]
        else:
            plan = _equal_chunks(n)
        plans.append(tuple(plan))
    plans = tuple(plans)
    nc = _get_compiled(plans)

    # --- per-core inputs (host lays everything partition-major) ---
    idx_all = np.concatenate(idx_list)
    xg = xf[idx_all].astype(BF16)                       # [NT, C]
    xt = np.ascontiguousarray(
        xg.reshape(NT, KC, P).transpose(2, 1, 0)).reshape(P, KC * NT)

    w1b = w1.astype(BF16)                               # [E, C, H]
    w2b = w2.astype(BF16)                               # [E, H, C]
    b1f = b1.astype(np.float32)

    in_maps = []
    for k in range(N_CORES):
        hs = slice(k * H8, (k + 1) * H8)
        w1r = np.ascontiguousarray(
            w1b[:, :, hs].reshape(E, KC, P, H8)
            .transpose(2, 0, 1, 3)).reshape(P, E * KC * H8)
        w2r = np.ascontiguousarray(
            w2b[:, hs, :].reshape(E, KH, P, C)
            .transpose(2, 0, 1, 3)).reshape(P, E * KH * C)
        b1r = np.ascontiguousarray(
            b1f[:, hs].reshape(E, KH, P).transpose(2, 0, 1)).reshape(P, E * KH)
        in_maps.append({"xt": xt, "w1r": w1r, "w2r": w2r, "b1r": b1r})

    try:
        res = bass_utils.run_bass_kernel_spmd(
            nc, in_maps, core_ids=list(range(N_CORES)), trace=TRACE)
    except Exception:
        if not TRACE:
            raise
        # profiling plumbing can fail in restricted environments — the
        # numerical result must not depend on it
        res = bass_utils.run_bass_kernel_spmd(
            nc, in_maps, core_ids=list(range(N_CORES)), trace=False)
    LAST_RESULTS = res
    LAST_EXEC_NS = res.exec_time_ns

    # --- combine (host unshard): sum the 8 H-slice partials, add b2,
    #     apply gate combine weights ---
    acc = np.zeros((P, KC, NT), dtype=np.float32)
    for k in range(N_CORES):
        acc += res.results[k]["outp"].reshape(P, KC, NT)
    y = np.ascontiguousarray(acc.transpose(2, 1, 0)).reshape(NT, C)

    out = np.zeros((N, C), dtype=np.float32)
    b2f = b2.astype(np.float32)
    pos = 0
    for e in range(E):
        n_e = counts[e]
        seg = y[pos:pos + n_e]
        seg += b2f[e][None, :]
        out[idx_list[e]] += cw_list[e][:, None] * seg
        pos += n_e
    return out.reshape(B, T, C).astype(x.dtype, copy=False)
